# revision 30
# baseline (speedup 1.0000x reference)
"""Trainium2 Bass kernel for nn_ChannelSpatialModulatedConv2d.

Data-parallel over batch across 8 NeuronCores (4 samples each). Per core:
  1. style  = style_chan @ (mod_w*ls).T + mod_b             (PE, fp32)
  2. wsc    = bf16(conv_scale * weight * style[b,ci])       (DVE tensor_scalar)
     demod  = rsqrt(sum(wsc^2) over (ci,kk) + eps) per co   (ACT square, DVE
              kk-reduce, PE ones-matmul -> [128co,1] partition-native)
  3. conv2d(x[b], wsc) via 18 accumulating bf16 matmuls per [128co x 512yx]
     PSUM tile (2 ci-tiles x 9 shifts), shifted-window APs over a zero-padded
     66x66 bf16 SBUF image. bf16 streams ~1 col/cycle and enables FWL weight
     loads (weights laid out [ci, kk, co] so each 128-co weight column is
     contiguous).
  4. sp map = style_sp @ (sp_w*ls).T + sp_b (bf16), spatially demodulated.
     The PSUM epilogue computes out = (psum * demod[co]) * spmap[yx] in one
     fused scalar_tensor_tensor writing bf16 (demod is NOT folded into the
     weights; conv is linear so this is equivalent). spmap rows are staged
     to DRAM in bf16 and broadcast to 128 partitions via DMA (engines and
     SBUF DMA sources reject 0-step partition APs).

Output is written to DRAM in bf16 and widened to fp32 on the host.

DMA queue split: x images on the sync queue, weights/sp_wT on sync early,
output stores on the scalar queue, small loads + spm staging + broadcasts
on the gpsimd (SWDGE, earliest-up) queue.

The baked walrus build only supports ONE sync wait per instruction, so the
Bass subclass rewrites the scheduled BIR JSON, hoisting extra waits onto
single-wait EventSemaphore carriers inserted before the instruction (same
engine => identical blocking semantics).
"""

import json
import sys
from contextlib import ExitStack

for _p in ("/opt/pypackages", "/opt/trn_rl_repo"):
    if _p not in sys.path:
        sys.path.insert(0, _p)

import ml_dtypes
import numpy as np

import concourse.bass as bass
import concourse.mybir as mybir
import concourse.tile as tile
from concourse.tile_rust import add_dep_helper
from concourse.bass_utils import run_bass_kernel_spmd

# Problem constants (hardcoded per harness contract)
B, CIN, COUT, K = 32, 256, 256, 3
STYLE_DIM, SP = 512, 64
EPS = 1e-6
LS = 1.0 / (STYLE_DIM // 2) ** 0.5      # EqualLinear scale = 1/16
CS = 1.0 / (CIN * K * K) ** 0.5         # conv fan-in scale = 1/48
N_CORES = 8
BPC = B // N_CORES                      # samples per core = 4
SPP = SP + 2                            # padded image dim = 66
CKK = COUT * K * K                      # 2304 free columns in weight layout
YX = SP * SP                            # 4096 spatial positions

F32 = mybir.dt.float32
F32R = mybir.dt.float32r
BF16 = mybir.dt.bfloat16
AF = mybir.ActivationFunctionType
ALU = mybir.AluOpType


def _split_multi_waits(bir: dict) -> int:
    """Hoist all but one sync wait from every instruction onto single-wait
    EventSemaphore carriers inserted immediately before it (same engine)."""
    ctr = 0
    for fn in bir.get("functions", []):
        for blk in fn.get("blocks", []):
            insts = blk.get("instructions", [])
            if not any(
                len(((i.get("sync_info") or {}).get("on_wait") or [])) > 1
                for i in insts
            ):
                continue
            new_insts = []
            for inst in insts:
                si = inst.get("sync_info")
                ow = (si or {}).get("on_wait") or []
                if len(ow) > 1:
                    for w in ow[:-1]:
                        ctr += 1
                        new_insts.append({
                            "debug": inst.get("debug", 0),
                            "engine": inst["engine"],
                            "ins": [],
                            "outs": [],
                            "name": f"waitsplit-{ctr}",
                            "opcode": "EventSemaphore",
                            "sync_info": {"on_update": [], "on_wait": [w]},
                        })
                    si["on_wait"] = [ow[-1]]
                new_insts.append(inst)
            blk["instructions"] = new_insts
    return ctr


class _WaitSplitBass(bass.Bass):
    def to_json_bytes(self) -> bytes:
        raw = super().to_json_bytes()
        bir = json.loads(raw)
        if _split_multi_waits(bir):
            return json.dumps(bir).encode()
        return raw


def _pbcast(ap, n):
    """Manual 0-step partition broadcast AP (DMA-only; engines reject it)."""
    return bass.AP(tensor=ap.tensor, offset=ap.offset,
                   ap=[[0, n]] + [list(d) for d in ap.ap[1:]])


def _build_program() -> bass.Bass:
    nc = _WaitSplitBass("TRN2", target_bir_lowering=False, debug=False)

    x_d = nc.dram_tensor("x", [BPC, CIN, SPP, SPP], BF16, kind="ExternalInput")
    stylecT_d = nc.dram_tensor("stylecT", [256, BPC], BF16, kind="ExternalInput")
    stylesT_d = nc.dram_tensor("stylesT", [256, BPC], BF16, kind="ExternalInput")
    wT_d = nc.dram_tensor("wT", [CIN, CKK], BF16, kind="ExternalInput")
    mod_wT_d = nc.dram_tensor("mod_wT", [256, CIN], BF16, kind="ExternalInput")
    mod_b_d = nc.dram_tensor("mod_b", [CIN, 1], F32, kind="ExternalInput")
    sp_wT_d = nc.dram_tensor("sp_wT", [256, YX], BF16, kind="ExternalInput")
    sp_b_d = nc.dram_tensor("sp_b", [1, YX], BF16, kind="ExternalInput")
    out_d = nc.dram_tensor("out", [BPC, COUT, SP, SP], BF16, kind="ExternalOutput")
    spm_d = nc.dram_tensor("spm_scratch", [BPC, YX], BF16, kind="Internal")
    dspt_d = nc.dram_tensor("dspt_scratch", [BPC, 1], F32, kind="Internal")

    NROW = 6            # x-image rows per DMA chunk (sample 0 fine-grained)
    NCH = SPP // NROW   # 11 chunks of 6 rows

    with tile.TileContext(nc) as tc:
        with tc.tile_pool(name="const", bufs=1) as cpool, \
             tc.tile_pool(name="persist", bufs=1) as ppool, \
             tc.tile_pool(name="sps", bufs=1, space="PSUM") as spsum:

            # ---------- constants ----------
            onesF = cpool.tile([128, 2], F32, name="onesF")
            nc.vector.memset(onesF, 1.0)
            ones = cpool.tile([128, 2], F32R, name="ones")
            nc.vector.tensor_copy(ones, onesF)
            eps4 = cpool.tile([BPC, 1], F32, name="eps4")
            nc.vector.memset(eps4, EPS)
            eps128 = cpool.tile([128, 1], F32, name="eps128")
            nc.vector.memset(eps128, EPS)

            # ---------- persistent weights / style ----------
            wt = [ppool.tile([128, CKK], BF16, name=f"wt{k}") for k in range(2)]
            stylec = [ppool.tile([128, BPC], F32, name=f"stylec{k}") for k in range(2)]
            spm = ppool.tile([BPC, YX], BF16, name="spm")
            _xp_cm = tc.tile_pool(name="xp", bufs=2)
            xppool = _xp_cm.__enter__()
            _sw_cm = tc.tile_pool(name="swtc", bufs=8)
            swpool = _sw_cm.__enter__()

            def load_xp(xp, b, fine):
                # row-chunked loads: early row blocks land first so early
                # conv groups can start while the rest streams
                src = x_d.ap()[b, :, :, :].rearrange("(k p) r c -> k p (r c)", k=2)
                if fine:
                    bounds = [(j * NROW * SPP, (j + 1) * NROW * SPP)
                              for j in range(NCH)]
                else:
                    half = (SPP // 2) * SPP
                    bounds = [(0, half), (half, SPP * SPP)]
                for lo, hi in bounds:
                    for k in range(2):
                        nc.scalar.dma_start(out=xp[k][:, lo:hi],
                                            in_=src[k, :, lo:hi])

            # ---------- setup (pool freed afterwards) ----------
            with tc.tile_pool(name="setup", bufs=1) as spool, \
                 tc.tile_pool(name="setup_ps", bufs=1, space="PSUM") as supsum:
                # ---------- PE warmup ----------
                # The HAM clock monitor holds the PE at 1.2 GHz until it has
                # been busy ~3.4us. Burn idle startup time (DMA queues come up
                # at ~2.6us SWDGE / ~9.5us HWDGE) on dependency-free matmuls
                # so the real style/conv matmuls run at 2.4 GHz from the start.
                wu = spool.tile([128, 512], BF16, name="wu")
                nc.vector.memset(wu, 0.0)
                wups = supsum.tile([128, 512], F32, name="wups", tag="wups")
                for i in range(25):
                    nc.tensor.matmul(wups, wu[:, 0:128], wu, start=True, stop=True)

                mw = [spool.tile([128, CIN], BF16, name=f"mw{k}") for k in range(2)]
                stc = [spool.tile([128, BPC], BF16, name=f"stc{k}") for k in range(2)]
                sts = [ppool.tile([128, BPC], BF16, name=f"sts{k}") for k in range(2)]
                mb = [spool.tile([128, 1], F32, name=f"mb{k}") for k in range(2)]
                spb = ppool.tile([BPC, YX], BF16, name="spb", tag="spsc")
                scratch = ppool.tile([BPC, YX], F32, name="scratch", tag="spsc2")
                # style-matmul gating loads first on the scalar HWDGE (fast;
                # the SWDGE comes up earlier but only moves ~10-20 GB/s and
                # was the old startup bottleneck)
                for k in range(2):
                    nc.scalar.dma_start(out=mw[k], in_=mod_wT_d.ap()[k * 128:(k + 1) * 128, :])
                    nc.scalar.dma_start(out=stc[k], in_=stylecT_d.ap()[k * 128:(k + 1) * 128, :])
                    nc.scalar.dma_start(out=mb[k], in_=mod_b_d.ap()[k * 128:(k + 1) * 128, :])
                for k in range(2):
                    nc.gpsimd.dma_start(out=sts[k], in_=stylesT_d.ap()[k * 128:(k + 1) * 128, :])
                nc.gpsimd.dma_start(out=spb, in_=_pbcast(sp_b_d.ap(), BPC))
                # conv weights (bf16, chunked so the first wsc chunk can start
                # as soon as its s-triple lands), interleaved with sample 0's
                # early image rows
                xp0 = [
                    xppool.tile([128, SPP * SPP], BF16, name=f"xp{k}_0", tag=f"xp{k}")
                    for k in range(2)
                ]
                xsrc0 = x_d.ap()[0, :, :, :].rearrange("(k p) r c -> k p (r c)", k=2)

                def xp0_chunk(j):
                    lo, hi = j * NROW * SPP, (j + 1) * NROW * SPP
                    for k in range(2):
                        nc.scalar.dma_start(out=xp0[k][:, lo:hi], in_=xsrc0[k, :, lo:hi])

                # sp_w column tiles: n0-3 on the scalar queue behind sample
                # 0's early rows, n4-7 on the sync queue behind the conv
                # weights, so the sp-map matmuls can run right behind the
                # first conv group
                swtc = [[swpool.tile([128, 512], BF16, name=f"swtc_{n}_{k}",
                                     tag=f"swtc{n}_{k}", bufs=1)
                         for k in range(2)] for n in range(8)]
                for j in range(4):
                    xp0_chunk(j)
                for n in range(4):
                    for k in range(2):
                        nc.scalar.dma_start(
                            out=swtc[n][k],
                            in_=sp_wT_d.ap()[k * 128:(k + 1) * 128,
                                             n * 512:(n + 1) * 512])
                for j in range(4, NCH):
                    xp0_chunk(j)
                for c in range(3):
                    for k in range(2):
                        nc.sync.dma_start(
                            out=wt[k][:, c * 768:(c + 1) * 768],
                            in_=wT_d.ap()[k * 128:(k + 1) * 128, c * 768:(c + 1) * 768])
                for n in range(4, 8):
                    for k in range(2):
                        nc.sync.dma_start(
                            out=swtc[n][k],
                            in_=sp_wT_d.ap()[k * 128:(k + 1) * 128,
                                             n * 512:(n + 1) * 512])

                # channel style: stylec[ci, b] = CS*(mod_w@chan*LS + mod_b)
                for m in range(2):
                    ps_style = supsum.tile([128, BPC], F32, name="ps_style", tag="ps_style")
                    for k in range(2):
                        nc.tensor.matmul(
                            ps_style, mw[k][:, m * 128:(m + 1) * 128], stc[k],
                            start=(k == 0), stop=(k == 1),
                        )
                    mbcs = spool.tile([128, 1], F32, name=f"mbcs{m}")
                    nc.scalar.mul(mbcs, mb[m], CS)
                    nc.scalar.activation(
                        out=stylec[m], in_=ps_style, func=AF.Identity,
                        bias=mbcs, scale=LS * CS,
                    )

                # spatial map: spm[b, yx] = bf16(sp_psum*LS + sp_b)
                # Per-chunk pipeline: matmul -> (bias+scale) -> DRAM staging of
                # the UNSCALED map -> fused square accumulation. The global
                # spatial demod factor is folded into the per-co demod column
                # (dcol) later, so nothing here serializes on the full map.
                sums = ppool.tile([BPC, 8], F32, name="sums")
                sp_mms = []
                for n in range(8):
                    ps_sp = spsum.tile([BPC, 512], F32, name="ps_sp", tag="ps_sp")
                    for k in range(2):
                        sp_mms.append(nc.tensor.matmul(
                            ps_sp, sts[k], swtc[n][k],
                            start=(k == 0), stop=(k == 1),
                        ))
                    with nc.allow_low_precision(reason="spatial map in bf16"):
                        nc.vector.scalar_tensor_tensor(
                            out=spm[:, n * 512:(n + 1) * 512], in0=ps_sp, scalar=LS,
                            in1=spb[:, n * 512:(n + 1) * 512],
                            op0=ALU.mult, op1=ALU.add,
                        )
                    nc.gpsimd.dma_start(
                        out=spm_d.ap()[:, n * 512:(n + 1) * 512],
                        in_=spm[:, n * 512:(n + 1) * 512],
                    )
                    nc.vector.scalar_tensor_tensor(
                        out=scratch[:, n * 512:(n + 1) * 512],
                        in0=spm[:, n * 512:(n + 1) * 512], scalar=1.0,
                        in1=spm[:, n * 512:(n + 1) * 512],
                        op0=ALU.mult, op1=ALU.mult,
                        accum_out=sums[:, n:n + 1],
                    )

                # global spatial demod scalar: dspt = sqrt(YX/sum + eps)
                ssq = ppool.tile([BPC, 1], F32, name="ssq")
                nc.vector.reduce_sum(out=ssq, in_=sums, axis=mybir.AxisListType.X)
                rsq = ppool.tile([BPC, 1], F32, name="rsq")
                nc.vector.reciprocal(rsq, ssq)
                dspt = ppool.tile([BPC, 1], F32, name="dspt")
                nc.scalar.activation(
                    out=dspt, in_=rsq, func=AF.Sqrt, bias=eps4, scale=float(YX),
                )
                nc.gpsimd.dma_start(out=dspt_d.ap(), in_=dspt)

            # ---------- per-sample pipeline ----------
            _stack = ExitStack()
            cpsum = _stack.enter_context(tc.tile_pool(name="cps", bufs=6, space="PSUM"))
            wscpool = _stack.enter_context(tc.tile_pool(name="wsc", bufs=2))
            wsqpool = _stack.enter_context(tc.tile_pool(name="wsq", bufs=1))
            dempool = _stack.enter_context(tc.tile_pool(name="dem", bufs=2))
            opool = _stack.enter_context(tc.tile_pool(name="ot", bufs=3))
            smpool = _stack.enter_context(tc.tile_pool(name="smb", bufs=2))

            for b in range(BPC):
                # modulated (pre-demod) weight: wsc = bf16(wt * (CS*style[ci,b]))
                # layout [ci, kk, co] so weight columns are contiguous (FWL).
                # Chunked (s-triples) so sample 0's first conv matmuls unlock
                # after 1/3 of the DVE work.
                wsc = [
                    wscpool.tile([128, CKK], BF16, name=f"wsc{k}_{b}", tag=f"wsc{k}")
                    for k in range(2)
                ]
                wsq = [
                    wsqpool.tile([128, CKK], BF16, name=f"wsq{k}_{b}", tag=f"wsq{k}")
                    for k in range(2)
                ]
                wsqk = [
                    dempool.tile([128, COUT], F32R, name=f"wsqk{k}_{b}", tag=f"wsqk{k}")
                    for k in range(2)
                ]
                for c in range(3):
                    for k in range(2):
                        sl = slice(c * 768, (c + 1) * 768)
                        with nc.allow_low_precision(reason="conv runs in bf16"):
                            nc.vector.tensor_scalar_mul(
                                wsc[k][:, sl], wt[k][:, sl], stylec[k][:, b:b + 1])
                        nc.scalar.activation(out=wsq[k][:, sl], in_=wsc[k][:, sl],
                                             func=AF.Square)
                for k in range(2):
                    with nc.allow_low_precision(reason="f32r is fp32-width"):
                        nc.vector.reduce_sum(
                            out=wsqk[k],
                            in_=wsq[k].rearrange("p (kk co) -> p co kk", kk=9),
                            axis=mybir.AxisListType.X,
                        )

                # per-sample spatial demod scalar, replicated to 128 partitions
                dsptb = dempool.tile([128, 1], F32, name=f"dsptb_{b}", tag="dsptb")
                nc.gpsimd.dma_start(out=dsptb, in_=_pbcast(dspt_d.ap()[b:b + 1, :], 128))

                # per-co demod, partition-native: ps_d[co,1] = sum_ci wsqk
                dcol = []
                for m in range(2):
                    ps_d = spsum.tile([128, 2], F32, name=f"ps_d_{b}_{m}", tag="ps_d")
                    for k in range(2):
                        nc.tensor.matmul(
                            ps_d, wsqk[k][:, m * 128:(m + 1) * 128], ones,
                            start=(k == 0), stop=(k == 1),
                        )
                    dsq = dempool.tile([128, 1], F32, name=f"dsq_{b}_{m}", tag=f"dsq{m}")
                    nc.scalar.activation(out=dsq, in_=ps_d[:, 0:1], func=AF.Sqrt,
                                         bias=eps128, scale=1.0)
                    dc = dempool.tile([128, 1], F32, name=f"dcol_{b}_{m}", tag=f"dcol{m}")
                    nc.vector.reciprocal(dc, dsq)
                    nc.vector.tensor_mul(dc, dc, dsptb)
                    dcol.append(dc)

                # padded input image [128ci, 66, 66] per ci-tile
                if b == 0:
                    xp = xp0
                else:
                    xp = [
                        xppool.tile([128, SPP * SPP], BF16, name=f"xp{k}_{b}", tag=f"xp{k}")
                        for k in range(2)
                    ]
                    load_xp(xp, b, fine=False)

                # conv + fused epilogue: out = bf16((psum * demod[co]) * spmap[yx])
                for n in range(8):
                    smb = smpool.tile([128, 512], BF16, name=f"smb_{b}_{n}", tag="smb")
                    nc.sync.dma_start(
                        out=smb,
                        in_=_pbcast(spm_d.ap()[b:b + 1, n * 512:(n + 1) * 512], 128),
                    )
                    for m in range(2):
                        ps = cpsum.tile([128, 512], F32, name=f"ps_{b}_{m}_{n}", tag="ps")
                        i = 0
                        for k in range(2):
                            wv = wsc[k].rearrange("p (kk co) -> p kk co", kk=9)
                            xpv = xp[k].rearrange("p (r c) -> p r c", c=SPP)
                            for s in range(9):
                                dy, dx = s // 3, s % 3
                                mm = nc.tensor.matmul(
                                    ps,
                                    wv[:, s, m * 128:(m + 1) * 128],
                                    xpv[:, n * 8 + dy:n * 8 + dy + 8, dx:dx + SP],
                                    start=(i == 0), stop=(i == 17),
                                )
                                i += 1
                        if b == 0 and n == 0 and m == 0 and sp_mms:
                            # Keep the spatial-map matmuls out of the PE
                            # stream until sample-0 conv is well underway
                            # (their sp_wT input streams in slowly; scheduling
                            # them early head-of-line-blocks the PE).
                            for _sp in sp_mms:
                                add_dep_helper(
                                    _sp.ins, mm.ins, sync=False,
                                    reason="sp-map after early sample-0 conv",
                                )
                            sp_mms = []
                        ot = opool.tile([128, 512], BF16, name=f"ot_{b}_{m}_{n}", tag="ot")
                        with nc.allow_low_precision(reason="output in bf16"):
                            nc.vector.scalar_tensor_tensor(
                                out=ot, in0=ps, scalar=dcol[m][:, 0:1], in1=smb,
                                op0=ALU.mult, op1=ALU.mult,
                            )
                        nc.sync.dma_start(
                            out=out_d.ap()[b, m * 128:(m + 1) * 128, n * 8:(n + 1) * 8, :],
                            in_=ot.rearrange("p (r c) -> p r c", c=SP),
                        )
            _stack.close()
            _sw_cm.__exit__(None, None, None)
            _xp_cm.__exit__(None, None, None)
    return nc


_prog_cache = {}


def _get_program() -> bass.Bass:
    if "nc" not in _prog_cache:
        _prog_cache["nc"] = _build_program()
    return _prog_cache["nc"]


def _make_in_maps(inputs):
    x = np.asarray(inputs["x"], dtype=np.float32)
    x = np.pad(x, ((0, 0), (0, 0), (1, 1), (1, 1)))
    x = x.astype(ml_dtypes.bfloat16)
    style_in = np.asarray(inputs["style_in"], dtype=np.float32)
    weight = np.asarray(inputs["weight"], dtype=np.float32)
    mod_w = np.asarray(inputs["mod_w"], dtype=np.float32)
    mod_b = np.asarray(inputs["mod_b"], dtype=np.float32)
    sp_w = np.asarray(inputs["sp_w"], dtype=np.float32)
    sp_b = np.asarray(inputs["sp_b"], dtype=np.float32)

    # replicated parameter layouts (pure transposes/reshapes + bf16 casts)
    wT = np.ascontiguousarray(
        weight[0].transpose(1, 2, 3, 0).reshape(CIN, CKK)
    ).astype(ml_dtypes.bfloat16)                                  # [ci, kk*co]
    mod_wT = np.ascontiguousarray(mod_w.T).astype(ml_dtypes.bfloat16)  # [sd, ci]
    mod_b2 = np.ascontiguousarray(mod_b.reshape(CIN, 1))
    sp_wT = np.ascontiguousarray(sp_w.T).astype(ml_dtypes.bfloat16)  # [sd, yx]
    sp_b2 = np.ascontiguousarray(sp_b.reshape(1, YX)).astype(ml_dtypes.bfloat16)

    in_maps = []
    for c in range(N_CORES):
        sl = slice(c * BPC, (c + 1) * BPC)
        st = style_in[sl].T
        in_maps.append({
            "x": np.ascontiguousarray(x[sl]),
            "stylecT": np.ascontiguousarray(st[:256]).astype(ml_dtypes.bfloat16),
            "stylesT": np.ascontiguousarray(st[256:]).astype(ml_dtypes.bfloat16),
            "wT": wT,
            "mod_wT": mod_wT,
            "mod_b": mod_b2,
            "sp_wT": sp_wT,
            "sp_b": sp_b2,
        })
    return in_maps


def _run(inputs, trace=False):
    nc = _get_program()
    in_maps = _make_in_maps(inputs)
    res = run_bass_kernel_spmd(nc, in_maps, core_ids=list(range(N_CORES)), trace=trace)
    out = np.concatenate(
        [np.asarray(res.results[c]["out"]).astype(np.float32) for c in range(N_CORES)],
        axis=0)
    return out, res


def kernel(**inputs) -> np.ndarray:
    out, _ = _run(inputs, trace=False)
    return out
